# revision 1
# baseline (speedup 1.0000x reference)
"""LoCon1d (position-specific conv1d) Trainium2 kernel.

out[b,o,s] = sum_{c,k} xpad[b,c,s+k] * w[o,c,s,k] + bias[o,s]
shapes: x (16,64,1024) f32, w (64,64,1024,3) f32, bias (64,1024) f32.

Sharding: sequence-parallel over 8 cores, 128 positions each.
Per-core mapping: positions split into two half-blocks (j, 64+j) packed
block-diagonally into the 128-partition contraction dim of the PE:
  stationary lhsT [128, 32]: rows 0:64 = x window (c) for pos j,
    cols 0:16; rows 64:128 = x window for pos 64+j, cols 16:32 (zeros
    elsewhere, baked in on host).
  moving rhs [128, 64]: rows 0:64 = w[o, c, j, k], rows 64:128 =
    w[o, c, 64+j, k] -> psum[0:16,o] = out(pos j), psum[16:32,o] =
    out(pos 64+j). 3 taps accumulate in PSUM.
All device tensors are host-side relayouts so DMAs are contiguous.
"""

import numpy as np

import concourse.bass as bass
import concourse.mybir as mybir
import concourse.tile as tile
from concourse import bacc, bass_utils

N_CORES = 8
B, CIN, COUT, S, K = 16, 64, 64, 1024, 3
SC = S // N_CORES          # positions per core (128)
H = SC // 2                # half-block (64)
JB = 16                    # position chunks per half-block
JI = H // JB               # positions per chunk (4)
TW = H + K - 1             # x window length per half-block (66)
XCH = 2                    # xr DMA split (t-dim chunks)

_DT = {"f32": mybir.dt.float32, "bf16": mybir.dt.bfloat16,
       "f16": mybir.dt.float16}

DTYPE = "f16"


def _np_dt(dt):
    if dt == "bf16":
        import ml_dtypes
        return ml_dtypes.bfloat16
    if dt == "f16":
        return np.float16
    return np.float32


def build_bass(dtype=DTYPE):
    dt = _DT[dtype]
    nc = bacc.Bacc("TRN2", target_bir_lowering=False, debug=False,
                   num_devices=N_CORES)
    xr = nc.dram_tensor("xr", [128, TW, 32], dt, kind="ExternalInput")
    wr = nc.dram_tensor("wr", [128, JB, JI, K, COUT], dt, kind="ExternalInput")
    br = nc.dram_tensor("br", [32, H, COUT], dt, kind="ExternalInput")
    out = nc.dram_tensor("out", [32, H, COUT], mybir.dt.float32,
                         kind="ExternalOutput")

    with tile.TileContext(nc) as tc:
        with (
            tc.tile_pool(name="xpool", bufs=1) as xpool,
            tc.tile_pool(name="wpool", bufs=JB) as wpool,
            tc.tile_pool(name="bpool", bufs=1) as bpool,
            tc.tile_pool(name="opool", bufs=4) as opool,
            tc.tile_pool(name="psum", bufs=8, space="PSUM") as pspool,
        ):
            # x first (every matmul needs it), in t-chunks so early
            # windows land quickly; then weight chunks; bias off-path.
            xr_sb = xpool.tile([128, TW, 32], dt)
            step = (TW + XCH - 1) // XCH
            for c in range(XCH):
                t0, t1 = c * step, min((c + 1) * step, TW)
                nc.sync.dma_start(out=xr_sb[:, t0:t1, :],
                                  in_=xr.ap()[:, t0:t1, :])
            br_sb = bpool.tile([32, H, COUT], dt)
            nc.gpsimd.dma_start(out=br_sb[:, :, :], in_=br.ap())
            w_t = []
            for jb in range(JB):
                wt = wpool.tile([128, JI, K, COUT], dt, tag="wt")
                eng = nc.sync if jb % 2 == 0 else nc.gpsimd
                eng.dma_start(out=wt[:, :, :, :],
                              in_=wr.ap()[:, jb, :, :, :])
                w_t.append(wt)

            for jb in range(JB):
                o_t = opool.tile([32, JI, COUT], mybir.dt.float32, tag="ot")
                for ji in range(JI):
                    j = jb * JI + ji
                    ps = pspool.tile([32, COUT], mybir.dt.float32, tag="ps")
                    for k in range(K):
                        nc.tensor.matmul(
                            ps[:, :],
                            lhsT=xr_sb[:, j + k, :],
                            rhs=w_t[jb][:, ji, k, :],
                            start=(k == 0),
                            stop=(k == K - 1),
                        )
                    nc.vector.tensor_add(
                        out=o_t[:, ji, :], in0=ps[:, :],
                        in1=br_sb[:, j, :],
                    )
                nc.sync.dma_start(out=out.ap()[:, jb * JI:(jb + 1) * JI, :],
                                  in_=o_t[:, :, :])
    nc.compile()
    return nc


def prep_inputs(input, weight, bias, dtype=DTYPE):
    """Host-side shard + relayout. Returns list of per-core input dicts."""
    npdt = _np_dt(dtype)
    xpad = np.pad(np.asarray(input, np.float32), ((0, 0), (0, 0), (1, 1)))
    w = np.asarray(weight, np.float32)
    bias = np.asarray(bias, np.float32)
    in_maps = []
    for i in range(N_CORES):
        s0 = i * SC
        # x: [p, b_ext, t] block-diagonal
        xa = xpad[:, :, s0:s0 + TW]             # (B, CIN, TW)
        xb = xpad[:, :, s0 + H:s0 + H + TW]
        xr = np.zeros((128, TW, 32), np.float32)
        xr[:64, :, :16] = xa.transpose(1, 2, 0)
        xr[64:, :, 16:] = xb.transpose(1, 2, 0)
        # w: [p(c+64*half), jb, ji, k, o]
        ws = w[:, :, s0:s0 + SC, :]             # (COUT, CIN, SC, K)
        wa = ws[:, :, :H, :].reshape(COUT, CIN, JB, JI, K)
        wb = ws[:, :, H:, :].reshape(COUT, CIN, JB, JI, K)
        wr = np.empty((128, JB, JI, K, COUT), np.float32)
        wr[:64] = wa.transpose(1, 2, 3, 4, 0)
        wr[64:] = wb.transpose(1, 2, 3, 4, 0)
        # bias: [b_ext, j, o] replicated over b
        bs = bias[:, s0:s0 + SC]                # (COUT, SC)
        br = np.empty((32, H, COUT), np.float32)
        br[:16] = bs[:, :H].T[None]
        br[16:] = bs[:, H:].T[None]
        in_maps.append({
            "xr": np.ascontiguousarray(xr.astype(npdt)),
            "wr": np.ascontiguousarray(wr.astype(npdt)),
            "br": np.ascontiguousarray(br.astype(npdt)),
        })
    return in_maps


def assemble_output(results):
    full = np.empty((B, COUT, S), np.float32)
    for i, r in enumerate(results):
        s0 = i * SC
        oc = r["out"]                            # (32, H, COUT)
        full[:, :, s0:s0 + H] = oc[:16].transpose(0, 2, 1)
        full[:, :, s0 + H:s0 + SC] = oc[16:].transpose(0, 2, 1)
    return full


_CACHED = {}


def run(inputs, dtype=DTYPE, trace=False):
    if dtype not in _CACHED:
        _CACHED[dtype] = build_bass(dtype)
    nc = _CACHED[dtype]
    in_maps = prep_inputs(inputs["input"], inputs["weight"], inputs["bias"],
                          dtype)
    res = bass_utils.run_bass_kernel_spmd(
        nc, in_maps, core_ids=list(range(N_CORES)), trace=trace)
    return assemble_output(res.results), res


def kernel(input, weight, bias):
    out, _ = run({"input": input, "weight": weight, "bias": bias},
                 trace=False)
    return out



# revision 6
# speedup vs baseline: 1.5148x; 1.5148x over previous
"""LoCon1d (position-specific conv1d) Trainium2 kernel.

out[b,o,s] = sum_{c,k} xpad[b,c,s+k] * w[o,c,s,k] + bias[o,s]
shapes: x (16,64,1024) f32, w (64,64,1024,3) f32, bias (64,1024) f32.

Sequence-parallel over 8 cores, 128 positions each, split into two
halves (A: j, B: 64+j). Per window t the PE stationary is the
block-diagonal x tile [128, 32] (rows 0:64 = half-A channels feeding
cols 0:16, rows 64:128 = half-B channels feeding cols 16:32). Moving
operand is the weight block for every (pair, tap) that consumes window
t, so one matmul covers up to 3 position-pairs x 64 out-channels and
taps accumulate in PSUM via a sliding window over 8-pair groups.

PSUM layout: 2 banks of [128, 512]; bank = 32 pairs as 4 col-tiled
group slots (tile_position col 32q) x 8 pair-slots x 64 channels. A
single [8,128] x [8,512] matmul per bank writes the bias into all 512
cols with start=True, so every later tap matmul is a pure accumulate.

Weights travel as float8e3 (e4 exp / m4 mantissa), x/bias as f16:
measured end-to-end rel err ~7e-3 vs the f32 reference.
"""

import numpy as np

import concourse.bass as bass
import concourse.mybir as mybir
import concourse.tile as tile
from concourse import bacc, bass_utils

N_CORES = 8
B, CIN, COUT, S, K = 16, 64, 64, 1024, 3
SC = S // N_CORES          # positions per core (128)
H = SC // 2                # half length (64)
NG = 8                     # pair groups per core (8 pairs each)
TW = H + K - 1             # windows per half (66)
GPC = 2                    # weight groups per DMA chunk
NCH = NG // GPC            # weight DMA chunks (4)
NWARM = 0                  # dummy PE warm-up matmuls

# per-group matmul column offsets: block i covers pair-slots lo..hi
_BLK = []
_cofs = 0
for _i in range(10):
    _lo, _hi = max(0, _i - 2), min(7, _i)
    _BLK.append((_lo, _hi, _cofs))
    _cofs += 64 * (_hi - _lo + 1)
GCOLS = _cofs              # 1536

W_DT = "f8e3"

_DT = {"f16": mybir.dt.float16, "f8e3": mybir.dt.float8e3}


def _np_dt(dt):
    if dt == "f8e3":
        import ml_dtypes
        return ml_dtypes.float8_e3m4
    return np.float16


def build_bass(w_dt=W_DT):
    wdt = _DT[w_dt]
    f16 = mybir.dt.float16
    f32 = mybir.dt.float32
    nc = bacc.Bacc("TRN2", target_bir_lowering=False, debug=False,
                   num_devices=N_CORES)
    xq = nc.dram_tensor("xq", [128, B, TW], f16, kind="ExternalInput")
    wq = nc.dram_tensor("wq", [128, NG, GCOLS], wdt, kind="ExternalInput")
    bq = nc.dram_tensor("bq", [8, 2, 512], f16, kind="ExternalInput")
    ones = nc.dram_tensor("ones", [8, 128], f16, kind="ExternalInput")
    out = nc.dram_tensor("out", [2, 128, 512], f16, kind="ExternalOutput")

    with tile.TileContext(nc) as tc:
        with (
            tc.tile_pool(name="xpool", bufs=1) as xpool,
            tc.tile_pool(name="wpool", bufs=1) as wpool,
            tc.tile_pool(name="cpool", bufs=1) as cpool,
            tc.tile_pool(name="opool", bufs=1) as opool,
            tc.tile_pool(name="psum", bufs=1, space="PSUM") as pspool,
        ):
            ones_sb = cpool.tile([8, 128], f16, tag="ones")
            bias_sb = cpool.tile([8, 2, 512], f16, tag="bias")
            nc.gpsimd.dma_start(out=ones_sb[:, :], in_=ones.ap())
            nc.gpsimd.dma_start(out=bias_sb[:, :, :], in_=bq.ap())

            # block-diagonal stationary x: [p, b_ext, t]
            xr = xpool.tile([128, 2 * B, TW], f16, tag="xr")
            nc.gpsimd.memset(xr[0:64, B:2 * B, :], 0.0)
            nc.gpsimd.memset(xr[64:128, 0:B, :], 0.0)
            nc.sync.dma_start(out=xr[0:64, 0:B, :], in_=xq.ap()[0:64, :, :])
            nc.scalar.dma_start(out=xr[64:128, B:2 * B, :],
                                in_=xq.ap()[64:128, :, :])

            w_sb = []
            for ch in range(NCH):
                wt = wpool.tile([128, GPC, GCOLS], wdt, tag=f"wt{ch}")
                eng = nc.sync if ch % 2 == 0 else nc.scalar
                eng.dma_start(out=wt[:, :, :],
                              in_=wq.ap()[:, ch * GPC:(ch + 1) * GPC, :])
                w_sb.append(wt)

            ps = [pspool.tile([128, 512], f32, name=f"ps{b}", tag=f"ps{b}")
                  for b in range(2)]
            if NWARM:
                psw = pspool.tile([128, 512], f32, tag="psw")
                for _ in range(NWARM):
                    nc.tensor.matmul(psw[:, :], lhsT=ones_sb[:, :],
                                     rhs=bias_sb[:, 0, :],
                                     start=True, stop=True)
            for bank in range(2):
                nc.tensor.matmul(ps[bank][:, :], lhsT=ones_sb[:, :],
                                 rhs=bias_sb[:, bank, :],
                                 start=True, stop=False)

            ob = [opool.tile([128, 512], f16, name=f"ob{b}", tag=f"ob{b}")
                  for b in range(2)]
            for bank in range(2):
                for qp in range(2):            # col-slot pair (chunk) index
                    ch = 2 * bank + qp
                    for i in range(10):
                        lo, hi, cofs = _BLK[i]
                        wd = 64 * (hi - lo + 1)
                        for qq in range(2):
                            q = 2 * qp + qq
                            g = 4 * bank + q
                            t = 8 * g + i
                            last = (qp == 1 and i == 9 and qq == 1)
                            nc.tensor.matmul(
                                ps[bank][32 * q + 0:32 * q + 32,
                                         64 * lo:64 * (hi + 1)],
                                lhsT=xr[:, :, t],
                                rhs=w_sb[ch][:, qq, cofs:cofs + wd],
                                start=False, stop=last,
                                tile_position=(0, 32 * q),
                            )
                nc.vector.tensor_copy(out=ob[bank][:, :], in_=ps[bank][:, :])
                nc.gpsimd.dma_start(out=out.ap()[bank, :, :],
                                    in_=ob[bank][:, :])
    nc.compile()
    return nc


def prep_inputs(input, weight, bias, w_dt=W_DT):
    """Host-side shard + relayout. Returns list of per-core input dicts."""
    wnp = _np_dt(w_dt)
    xpad = np.pad(np.asarray(input, np.float32), ((0, 0), (0, 0), (1, 1)))
    w = np.asarray(weight, np.float32).transpose(1, 2, 3, 0)  # (c, s, k, o)
    bias = np.asarray(bias, np.float32)

    ones = np.zeros((8, 128), np.float16)
    for r in range(8):
        m0 = 32 * (r // 2) + 16 * (r % 2)
        ones[r, m0:m0 + 16] = 1.0

    in_maps = []
    for core in range(N_CORES):
        s0 = core * SC
        xq = np.empty((128, B, TW), np.float16)
        xq[0:64] = xpad[:, :, s0:s0 + TW].transpose(1, 0, 2)
        xq[64:128] = xpad[:, :, s0 + H:s0 + H + TW].transpose(1, 0, 2)

        wq = np.empty((128, NG, GCOLS), np.float32)
        for g in range(NG):
            for i in range(10):
                lo, hi, cofs = _BLK[i]
                for slot in range(lo, hi + 1):
                    j = 8 * g + slot
                    k = i - slot
                    c0 = cofs + (slot - lo) * 64
                    wq[0:64, g, c0:c0 + 64] = w[:, s0 + j, k, :]
                    wq[64:128, g, c0:c0 + 64] = w[:, s0 + H + j, k, :]

        bq = np.empty((8, 2, 512), np.float16)
        for r in range(8):
            q, half = r // 2, r % 2
            for bank in range(2):
                sl = s0 + 32 * bank + 8 * q + 64 * half
                bq[r, bank] = bias[:, sl:sl + 8].T.reshape(512)

        in_maps.append({
            "xq": np.ascontiguousarray(xq),
            "wq": np.ascontiguousarray(wq.astype(wnp)),
            "bq": bq,
            "ones": ones,
        })
    return in_maps


def assemble_output(results):
    full = np.empty((B, COUT, S), np.float32)
    for core, r in enumerate(results):
        s0 = core * SC
        oc = np.asarray(r["out"], np.float32)     # (2, 128, 512)
        oc = oc.reshape(2, 4, 2, B, 8, COUT)      # bank q half b slot o
        oc = oc.transpose(3, 5, 2, 0, 1, 4)       # b o half bank q slot
        full[:, :, s0:s0 + SC] = oc.reshape(B, COUT, SC)
    return full


_CACHED = {}


def run(inputs, w_dt=W_DT, trace=False):
    if w_dt not in _CACHED:
        _CACHED[w_dt] = build_bass(w_dt)
    nc = _CACHED[w_dt]
    in_maps = prep_inputs(inputs["input"], inputs["weight"], inputs["bias"],
                          w_dt)
    res = bass_utils.run_bass_kernel_spmd(
        nc, in_maps, core_ids=list(range(N_CORES)), trace=trace)
    return assemble_output(res.results), res


def kernel(input, weight, bias):
    out, _ = run({"input": input, "weight": weight, "bias": bias},
                 trace=False)
    return out
